# revision 5
# baseline (speedup 1.0000x reference)
"""Trainium2 Bass kernel for GNN message-passing attention MLP.

Computation (per node b with N=32 neighbors, F=128 features):
  h_nb   = relu(input1 @ W_nb + b_nb)          [B,N,H1]
  h_self = relu(input2 @ W_self + b_self)      [B,H1]
  z      = relu(h_nb @ W_a1[:H1] + h_self @ W_a1[H1:] + b_a1)   [B,N,H2]
  out    = (z @ W_a2 + b_a2).reshape(B*N, 1)

Strategy: data-parallel over 8 NeuronCores (6250 nodes each, padded to
6272).  Host-side prep casts input1 to fp8-e3m4 (the 4-bit mantissa
keeps the end-to-end rel-err ~1.5e-2, under the 2e-2 gate, while
halving HBM traffic) and pre-transposes to [F, neighbor, node] layout;
input2 stays bf16.  On device, nodes ride the matmul free dim (512-wide
superblocks).  mm1 computes h for neighbor pairs stacked on the K dim
(2-way column-tile concurrency, two pairs per 2-bank PSUM tile so the
relu+cast runs as one [128,1024] op).  mm2 consumes a full stacked pair
per matmul (K=128, M=32: z for 2 neighbors per instruction) with 4-way
column-tile concurrency, the self path accumulating via one
replicated-weight matmul per 8 neighbors; mm2 trails mm1 by two blocks
so the PE never waits on the relu engines.  The final H2-contraction
accumulates all 32 neighbors into a [32, nodes] PSUM tile; the
node-major transpose of the output happens on the host.  Input DMA
alternates between the sync and scalar hardware DGE queues to double
descriptor-processing throughput.
"""

import sys
import types

import numpy as np
import ml_dtypes

import concourse.bass as bass
import concourse.mybir as mybir
from concourse import bacc
from concourse.tile import TileContext
from concourse.bass_utils import run_bass_kernel_spmd


def _ensure_axon_hooks():
    """bass_utils' trace path imports antenv.axon_hooks, which this image
    lacks; synthesize it (wired to the PJRT plugin's NTFF profiler) so a
    BASS_TRACE=1 environment doesn't crash the run."""
    try:
        import antenv.axon_hooks  # noqa: F401
        return
    except ImportError:
        pass
    try:
        import antenv
        mod = types.ModuleType("antenv.axon_hooks")
        holder = [None]
        mod.set_axon_ntff_profile_hook = lambda h: holder.__setitem__(0, h)
        mod.get_axon_ntff_profile_hook = lambda: holder[0]
        sys.modules["antenv.axon_hooks"] = mod
        antenv.axon_hooks = mod
        try:
            from trn_agent_boot.trn_boot import _ntff_profile_via_ctypes
            mod.set_axon_ntff_profile_hook(
                _ntff_profile_via_ctypes("/opt/axon/libaxon_pjrt.so"))
        except Exception:
            pass
    except Exception:
        pass


_ensure_axon_hooks()

BF16 = ml_dtypes.bfloat16
E3M4 = ml_dtypes.float8_e3m4

B, N, F = 50000, 32, 128
H1, H2 = 64, 16
N_CORES = 8
B_SH = B // N_CORES            # 6250 nodes per core
B_PAD = 6272                   # padded to 49*128
SB = 512                       # superblock: nodes per compute block
SBS = [(s * SB, SB) for s in range(B_PAD // SB)]
_rem = B_PAD - (B_PAD // SB) * SB
if _rem:
    SBS.append(((B_PAD // SB) * SB, _rem))
R_SH = B_SH * N                # valid rows per core (200000)

_cache = {}
last_results = None  # BassKernelResults of the most recent run (for test harness)
TRACE = False        # set True from test harness to capture an HW profile


def _build_graph():
    dt = mybir.dt
    nc = bacc.Bacc("TRN2", target_bir_lowering=False, debug=False,
                   num_devices=N_CORES)

    xt = nc.declare_dram_parameter("xt", [128 * N * B_PAD], dt.float8e3, isOutput=False)
    x2t = nc.declare_dram_parameter("x2t", [128 * B_PAD], dt.bfloat16, isOutput=False)
    wnb = nc.declare_dram_parameter("wnb", [128, H1], dt.bfloat16, isOutput=False)
    wself = nc.declare_dram_parameter("wself", [128, H1], dt.bfloat16, isOutput=False)
    w2s = nc.declare_dram_parameter("w2s", [128, 32], dt.bfloat16, isOutput=False)
    wrep = nc.declare_dram_parameter("wrep", [H1, 128], dt.bfloat16, isOutput=False)
    wg = nc.declare_dram_parameter("wg", [4, 128, 32], dt.bfloat16, isOutput=False)
    bnb = nc.declare_dram_parameter("bnb", [128, 1], dt.float32, isOutput=False)
    bself = nc.declare_dram_parameter("bself", [H1, 1], dt.float32, isOutput=False)
    bz = nc.declare_dram_parameter("bz", [128, 1], dt.float32, isOutput=False)
    out = nc.declare_dram_parameter("out", [32 * B_PAD], dt.bfloat16, isOutput=True)
    outv = out.rearrange("(r n) -> r n", r=32)

    RELU = mybir.ActivationFunctionType.Relu

    with TileContext(nc) as tc:
        with tc.tile_pool(name="const", bufs=1) as cpool, \
             tc.tile_pool(name="xp", bufs=3) as xpool, \
             tc.tile_pool(name="hp", bufs=8) as hpool, \
             tc.tile_pool(name="zs", bufs=8) as zpool, \
             tc.tile_pool(name="wp", bufs=2) as wpool, \
             tc.tile_pool(name="psum", bufs=1, space="PSUM") as ppool:
            # PSUM budget (8 banks): hp x2 tiles of 2 banks (mm1/self/
            # warmup), zp x2, wa x2.

            wnb_sb = cpool.tile([128, H1], dt.bfloat16)
            nc.scalar.dma_start(out=wnb_sb[:], in_=wnb[:])
            wself_sb = cpool.tile([128, H1], dt.bfloat16)
            nc.scalar.dma_start(out=wself_sb[:], in_=wself[:])
            w2s_sb = cpool.tile([128, 32], dt.bfloat16)
            nc.scalar.dma_start(out=w2s_sb[:], in_=w2s[:])
            wrep_sb = cpool.tile([H1, 128], dt.bfloat16)
            nc.scalar.dma_start(out=wrep_sb[:], in_=wrep[:])
            wg_sb = cpool.tile([128, 4, 32], dt.bfloat16)
            nc.scalar.dma_start(out=wg_sb[:], in_=wg.rearrange("g p m -> p g m"))
            bnb_sb = cpool.tile([128, 1], dt.float32)
            nc.scalar.dma_start(out=bnb_sb[:], in_=bnb[:])
            bself_sb = cpool.tile([H1, 1], dt.float32)
            nc.scalar.dma_start(out=bself_sb[:], in_=bself[:])
            bz_sb = cpool.tile([128, 1], dt.float32)
            nc.scalar.dma_start(out=bz_sb[:], in_=bz[:])

            first = True
            pend_blk = []   # mm2 blocks awaiting flush (2-block skew)
            pend_wa = None  # (z_sbs, ns) awaiting final contraction
            pend_out = None  # (wa_ps, n0, ns) awaiting copy + store

            def flush_blk(blk):
                # One zp tile per 8 neighbors: 4 K=128/M=32 matmuls at
                # the 4 column strips (concurrent in the PE), then one
                # replicated-weight matmul adds the self path everywhere.
                (ha, hb), ns, z_sbs, hself_sb = blk
                zp = ppool.tile([128, SB], dt.float32, tag="zp", bufs=2,
                                name="zp")
                for s in range(4):
                    h_sb = (ha, hb)[s // 2]
                    nc.tensor.matmul(
                        zp[32 * s: 32 * s + 32, :ns],
                        w2s_sb[:],
                        h_sb[:, s % 2, :ns],
                        start=True, stop=False,
                        skip_group_check=True,
                        tile_position=(0, 32 * s),
                    )
                nc.tensor.matmul(zp[:, :ns], wrep_sb[:],
                                 hself_sb[:, :ns],
                                 start=False, stop=True,
                                 skip_group_check=True)
                z_sb = zpool.tile([128, SB], dt.bfloat16, tag="z")
                if len(z_sbs) % 4 == 0:
                    nc.scalar.activation(z_sb[:, :ns], zp[:, :ns], RELU,
                                         bias=bz_sb[:], scale=1.0)
                else:
                    nc.vector.tensor_scalar(
                        z_sb[:, :ns], zp[:, :ns], bz_sb[:], 0.0,
                        mybir.AluOpType.add, mybir.AluOpType.max)
                z_sbs.append(z_sb)

            def emit_wa(w):
                # final contraction: 4 serial accumulating matmuls place
                # neighbor j of the 8-neighbor group t on psum row 8t+j
                z_sbs, ns = w
                wa_ps = ppool.tile([32, SB], dt.float32, tag="wa", bufs=2,
                                   name="wa_ps")
                for t in range(4):
                    nc.tensor.matmul(wa_ps[:, :ns],
                                     wg_sb[:, t, :], z_sbs[t][:, :ns],
                                     start=(t == 0), stop=(t == 3),
                                     skip_group_check=True)
                return wa_ps

            def emit_out(o):
                # psum -> bf16 sbuf -> HBM in [neighbor, node] layout;
                # the host does the final node-major transpose.
                wa_ps, on0, ons = o
                was_sb = wpool.tile([32, SB], dt.bfloat16, tag="was")
                nc.vector.tensor_copy(out=was_sb[:, :ons], in_=wa_ps[:, :ons])
                nc.scalar.dma_start(out=outv[:, on0: on0 + ons],
                                    in_=was_sb[:, :ons])

            for n0, ns in SBS:
                # -- inputs for this superblock (pre-transposed on host),
                #    split into 4 chunks of 8 neighbors alternating between
                #    the sync and scalar DMA queues --
                x2_sb = xpool.tile([128, SB], dt.bfloat16, tag="x2")
                nc.sync.dma_start(
                    out=x2_sb[:, :ns],
                    in_=x2t[128 * n0: 128 * (n0 + ns)].rearrange(
                        "(f n) -> f n", f=128),
                )
                x_sb = xpool.tile([128, N * SB], dt.float8e3, tag="x")
                xt_sb = xt[128 * N * n0: 128 * N * (n0 + ns)].rearrange(
                    "(f j n) -> f j n", f=128, j=N)
                for ci, (j0, j1) in enumerate([(0, 8), (8, 16), (16, 24), (24, 32)]):
                    eng = nc.sync if ci % 2 == 0 else nc.scalar
                    eng.dma_start(
                        out=x_sb[:, j0 * ns: j1 * ns].rearrange(
                            "p (j n) -> p j n", j=j1 - j0),
                        in_=xt_sb[:, j0: j1, :],
                    )

                if first:
                    # HAM warm-up: ~3.5us of dense matmul right after the
                    # first DMA lands, so the PE clock-gate opens to 2.4GHz
                    # before the real stream starts.
                    first = False
                    warm = ppool.tile([128, 2, SB], dt.float32, tag="hp",
                                      bufs=2)
                    for _ in range(8):
                        nc.tensor.matmul(warm[0:H1, 0, :ns], wnb_sb[:],
                                         x2_sb[:, :ns], start=True, stop=True)

                # -- self path: h_self = relu(W_self.T @ x2T + b_self) --
                hs_psum = ppool.tile([128, 2, SB], dt.float32, tag="hp",
                                     bufs=2)
                nc.tensor.matmul(hs_psum[0:H1, 0, :ns], wself_sb[:],
                                 x2_sb[:, :ns], start=True, stop=True)
                hself_sb = hpool.tile([H1, SB], dt.bfloat16, tag="hself")
                nc.scalar.activation(hself_sb[:, :ns], hs_psum[0:H1, 0, :ns],
                                     RELU, bias=bself_sb[:], scale=1.0)

                z_sbs = []
                # mm1: two neighbor pairs per 2-bank psum tile (2-way
                # column-tile concurrency per pair), one [128,1024]
                # relu+cast per tile split across the scalar and vector
                # engines.  mm2/wa/store for older blocks interleave at
                # fixed points so every engine stays 2 blocks behind mm1.
                for bt in range(8):
                    hp = ppool.tile([128, 2, SB], dt.float32, tag="hp",
                                    bufs=2)
                    for u in range(2):
                        for c in range(2):
                            j = 4 * bt + 2 * u + c
                            nc.tensor.matmul(
                                hp[H1 * c: H1 * (c + 1), u, :ns],
                                wnb_sb[:],
                                x_sb[:, j * ns: (j + 1) * ns],
                                start=True, stop=True,
                                tile_position=(0, H1 * c),
                            )
                    h_sb = hpool.tile([128, 2, SB], dt.bfloat16, tag="h")
                    for u in range(2):
                        if (2 * bt + u) % 16 in (0, 2, 4, 6, 8, 10, 12, 14, 15):
                            nc.scalar.activation(h_sb[:, u, :ns],
                                                 hp[:, u, :ns],
                                                 RELU, bias=bnb_sb[:],
                                                 scale=1.0)
                        else:
                            nc.vector.tensor_scalar(
                                h_sb[:, u, :ns], hp[:, u, :ns],
                                bnb_sb[:], 0.0,
                                mybir.AluOpType.add, mybir.AluOpType.max)

                    if bt % 2 == 1:
                        pend_blk.append(
                            ((prev_h, h_sb), ns, z_sbs, hself_sb))
                        if len(pend_blk) > 2:
                            flush_blk(pend_blk.pop(0))
                    else:
                        prev_h = h_sb

                    if bt == 3 and pend_wa is not None:
                        pend_out = (emit_wa(pend_wa), pend_wa_n0, pend_wa_ns)
                        pend_wa = None
                    if bt == 5 and pend_out is not None:
                        emit_out(pend_out)
                        pend_out = None

                pend_wa = (z_sbs, ns)
                pend_wa_n0, pend_wa_ns = n0, ns

            # drain the pipeline
            while pend_blk:
                flush_blk(pend_blk.pop(0))
            if pend_wa is not None:
                pend_out = (emit_wa(pend_wa), pend_wa_n0, pend_wa_ns)
            emit_out(pend_out)

    nc.compile()
    return nc


def _prep_weights(W_nb, b_nb, W_self, b_self, W_a1, b_a1, W_a2, b_a2):
    """Pack the dense weights into the layouts the kernel expects."""
    W_a1a = W_a1[:H1]          # [64, 16]
    W_a1b = W_a1[H1:]          # [64, 16]

    # mm2 stationary: block-diagonal so one matmul emits z for the two
    # K-stacked neighbors of an h tile.
    w2s = np.zeros((128, 32), np.float32)
    w2s[:H1, :H2] = W_a1a
    w2s[H1:, H2:] = W_a1a

    # self path replicated into all 8 16-column slots
    wrep = np.zeros((H1, 128), np.float32)
    for s in range(4):
        wrep[:, 32 * s: 32 * s + H2] = W_a1b
        wrep[:, 32 * s + 16: 32 * s + 16 + H2] = W_a1b

    # final contraction: z tile t holds neighbors 8t..8t+7; neighbor
    # 8t+2s sits on partitions 32s..32s+15, 8t+2s+1 on 32s+16..32s+31.
    # Place neighbor j on output row 8t+j mod 8 -> global row 8t+...
    wg = np.zeros((4, 128, 32), np.float32)
    for t in range(4):
        for s in range(4):
            wg[t, 32 * s: 32 * s + H2, 8 * t + 2 * s] = W_a2[:, 0]
            wg[t, 32 * s + 16: 32 * s + 16 + H2, 8 * t + 2 * s + 1] = W_a2[:, 0]

    bnb = np.concatenate([b_nb, b_nb]).reshape(128, 1).astype(np.float32)
    bselfv = b_self.reshape(H1, 1).astype(np.float32)
    bzv = np.zeros((128, 1), np.float32)
    for s in range(4):
        bzv[32 * s: 32 * s + H2, 0] = b_a1
        bzv[32 * s + 16: 32 * s + 16 + H2, 0] = b_a1

    return {
        "wnb": np.asarray(W_nb, np.float32).astype(BF16),
        "wself": np.asarray(W_self, np.float32).astype(BF16),
        "w2s": w2s.astype(BF16),
        "wrep": wrep.astype(BF16),
        "wg": wg.astype(BF16),
        "bnb": bnb,
        "bself": bselfv,
        "bz": bzv,
    }


def _prep_core_inputs(x_core, x2_core):
    """Pad to B_PAD nodes, cast (fp8 for x1, bf16 for x2), pre-transpose
    to [F, nbr, node] per superblock, and flatten."""
    xp = np.zeros((B_PAD, N, F), E3M4)
    xp[:B_SH] = x_core.astype(E3M4)
    x2p = np.zeros((B_PAD, F), BF16)
    x2p[:B_SH] = x2_core.astype(BF16)

    xt_parts = []
    x2t_parts = []
    for n0, ns in SBS:
        xt_parts.append(np.ascontiguousarray(
            xp[n0:n0 + ns].transpose(2, 1, 0)).reshape(-1))
        x2t_parts.append(np.ascontiguousarray(
            x2p[n0:n0 + ns].transpose(1, 0)).reshape(-1))
    return np.concatenate(xt_parts), np.concatenate(x2t_parts)


def kernel(input1, input2, W_nb, b_nb, W_self, b_self, W_a1, b_a1, W_a2, b_a2):
    global last_results
    if "nc" not in _cache:
        _cache["nc"] = _build_graph()
    nc = _cache["nc"]

    input1 = np.asarray(input1, np.float32)
    input2 = np.asarray(input2, np.float32)
    b_a2 = np.asarray(b_a2, np.float32)
    wmap = _prep_weights(
        np.asarray(W_nb, np.float32), np.asarray(b_nb, np.float32),
        np.asarray(W_self, np.float32), np.asarray(b_self, np.float32),
        np.asarray(W_a1, np.float32), np.asarray(b_a1, np.float32),
        np.asarray(W_a2, np.float32), b_a2)

    in_maps = []
    for c in range(N_CORES):
        xt_c, x2t_c = _prep_core_inputs(
            input1[c * B_SH: (c + 1) * B_SH],
            input2[c * B_SH: (c + 1) * B_SH])
        m = dict(wmap)
        m["xt"] = xt_c
        m["x2t"] = x2t_c
        in_maps.append(m)

    res = run_bass_kernel_spmd(nc, in_maps, core_ids=list(range(N_CORES)),
                               trace=TRACE)
    last_results = res

    out = np.empty((B * N, 1), np.float32)
    for c in range(N_CORES):
        # device output is [neighbor, node]; transpose to node-major here
        arr = res.results[c]["out"].reshape(32, B_PAD)[:, :B_SH]
        out[c * R_SH: (c + 1) * R_SH, 0] = (
            arr.astype(np.float32).T.reshape(-1) + b_a2[0])
    return out


# revision 9
# speedup vs baseline: 1.1908x; 1.1908x over previous
"""Trainium2 Bass kernel for GNN message-passing attention MLP.

Computation (per node b with N=32 neighbors, F=128 features):
  h_nb   = relu(input1 @ W_nb + b_nb)          [B,N,H1]
  h_self = relu(input2 @ W_self + b_self)      [B,H1]
  z      = relu(h_nb @ W_a1[:H1] + h_self @ W_a1[H1:] + b_a1)   [B,N,H2]
  out    = (z @ W_a2 + b_a2).reshape(B*N, 1)

Strategy: data-parallel over 8 NeuronCores (6250 nodes each, padded to
6272).  Host-side prep casts input1 to fp8-e3m4 (the 4-bit mantissa
keeps the end-to-end rel-err ~1.5e-2, under the 2e-2 gate, while
halving HBM traffic) and pre-transposes to [F, neighbor, node] layout;
input2 stays bf16.  On device, nodes ride the matmul free dim (512-wide
superblocks).  mm1 computes h for neighbor pairs stacked on the K dim
(2-way column-tile concurrency, two pairs per 2-bank PSUM tile so the
relu+cast runs as one [128,1024] op).  mm2 consumes a full stacked pair
per matmul (K=128, M=32: z for 2 neighbors per instruction) with 4-way
column-tile concurrency, the self path accumulating via one
replicated-weight matmul per 8 neighbors; mm2 trails mm1 by two blocks
so the PE never waits on the relu engines.  The final H2-contraction
accumulates all 32 neighbors into a [32, nodes] PSUM tile; the
node-major transpose of the output happens on the host.  Input DMA
alternates between the sync and scalar hardware DGE queues to double
descriptor-processing throughput.
"""

import sys
import types

import numpy as np
import ml_dtypes

import concourse.bass as bass
import concourse.mybir as mybir
from concourse import bacc
from concourse.tile import TileContext
from concourse.bass_utils import run_bass_kernel_spmd


def _ensure_axon_hooks():
    """bass_utils' trace path imports antenv.axon_hooks, which this image
    lacks; synthesize it (wired to the PJRT plugin's NTFF profiler) so a
    BASS_TRACE=1 environment doesn't crash the run."""
    try:
        import antenv.axon_hooks  # noqa: F401
        return
    except ImportError:
        pass
    try:
        import antenv
        mod = types.ModuleType("antenv.axon_hooks")
        holder = [None]
        mod.set_axon_ntff_profile_hook = lambda h: holder.__setitem__(0, h)
        mod.get_axon_ntff_profile_hook = lambda: holder[0]
        sys.modules["antenv.axon_hooks"] = mod
        antenv.axon_hooks = mod
        try:
            from trn_agent_boot.trn_boot import _ntff_profile_via_ctypes
            mod.set_axon_ntff_profile_hook(
                _ntff_profile_via_ctypes("/opt/axon/libaxon_pjrt.so"))
        except Exception:
            pass
    except Exception:
        pass


_ensure_axon_hooks()

BF16 = ml_dtypes.bfloat16
E3M4 = ml_dtypes.float8_e3m4

B, N, F = 50000, 32, 128
H1, H2 = 64, 16
N_CORES = 8
B_SH = B // N_CORES            # 6250 nodes per core
B_PAD = 6272                   # padded to 49*128
SB = 512                       # superblock: nodes per compute block
SBS = [(s * SB, SB) for s in range(B_PAD // SB)]
_rem = B_PAD - (B_PAD // SB) * SB
if _rem:
    SBS.append(((B_PAD // SB) * SB, _rem))
R_SH = B_SH * N                # valid rows per core (200000)

_cache = {}
last_results = None  # BassKernelResults of the most recent run (for test harness)
TRACE = False        # set True from test harness to capture an HW profile


def _build_graph():
    dt = mybir.dt
    nc = bacc.Bacc("TRN2", target_bir_lowering=False, debug=False,
                   num_devices=N_CORES)

    xt = nc.declare_dram_parameter("xt", [128 * N * B_PAD], dt.float8e3, isOutput=False)
    x2t = nc.declare_dram_parameter("x2t", [128 * B_PAD], dt.bfloat16, isOutput=False)
    wnb = nc.declare_dram_parameter("wnb", [128, H1], dt.bfloat16, isOutput=False)
    wself = nc.declare_dram_parameter("wself", [128, H1], dt.bfloat16, isOutput=False)
    w2s = nc.declare_dram_parameter("w2s", [128, 32], dt.bfloat16, isOutput=False)
    wrep = nc.declare_dram_parameter("wrep", [H1, 128], dt.bfloat16, isOutput=False)
    wg = nc.declare_dram_parameter("wg", [4, 128, 32], dt.bfloat16, isOutput=False)
    bnb = nc.declare_dram_parameter("bnb", [128, 1], dt.float32, isOutput=False)
    bself = nc.declare_dram_parameter("bself", [H1, 1], dt.float32, isOutput=False)
    bz = nc.declare_dram_parameter("bz", [128, 1], dt.float32, isOutput=False)
    out = nc.declare_dram_parameter("out", [32 * B_PAD], dt.bfloat16, isOutput=True)
    outv = out.rearrange("(r n) -> r n", r=32)

    RELU = mybir.ActivationFunctionType.Relu

    with TileContext(nc) as tc:
        with tc.tile_pool(name="const", bufs=1) as cpool, \
             tc.tile_pool(name="xp", bufs=3) as xpool, \
             tc.tile_pool(name="hp", bufs=14) as hpool, \
             tc.tile_pool(name="zs", bufs=8) as zpool, \
             tc.tile_pool(name="wp", bufs=2) as wpool, \
             tc.tile_pool(name="psum", bufs=1, space="PSUM") as ppool:
            # PSUM budget (8 banks): hp x2 tiles of 2 banks (mm1/self/
            # warmup), zp x2, wa x2.

            wnb_sb = cpool.tile([128, H1], dt.bfloat16)
            nc.scalar.dma_start(out=wnb_sb[:], in_=wnb[:])
            wself_sb = cpool.tile([128, H1], dt.bfloat16)
            nc.scalar.dma_start(out=wself_sb[:], in_=wself[:])
            w2s_sb = cpool.tile([128, 32], dt.bfloat16)
            nc.scalar.dma_start(out=w2s_sb[:], in_=w2s[:])
            wrep_sb = cpool.tile([H1, 128], dt.bfloat16)
            nc.scalar.dma_start(out=wrep_sb[:], in_=wrep[:])
            wg_sb = cpool.tile([128, 4, 32], dt.bfloat16)
            nc.scalar.dma_start(out=wg_sb[:], in_=wg.rearrange("g p m -> p g m"))
            bnb_sb = cpool.tile([128, 1], dt.float32)
            nc.scalar.dma_start(out=bnb_sb[:], in_=bnb[:])
            bself_sb = cpool.tile([H1, 1], dt.float32)
            nc.scalar.dma_start(out=bself_sb[:], in_=bself[:])
            bz_sb = cpool.tile([128, 1], dt.float32)
            nc.scalar.dma_start(out=bz_sb[:], in_=bz[:])

            first = True
            pend_blk = []   # mm2 blocks awaiting flush (2-block skew)
            pend_wa = None  # (z_sbs, ns) awaiting final contraction
            pend_out = None  # (wa_ps, n0, ns) awaiting copy + store

            def flush_blk(blk):
                # One zp tile per 8 neighbors: 4 K=128/M=32 matmuls at
                # the 4 column strips (concurrent in the PE), then one
                # replicated-weight matmul adds the self path everywhere.
                hs, ns, z_sbs, hself_sb = blk
                zp = ppool.tile([128, SB], dt.float32, tag="zp", bufs=2,
                                name="zp")
                for s in range(4):
                    nc.tensor.matmul(
                        zp[32 * s: 32 * s + 32, :ns],
                        w2s_sb[:],
                        hs[s][:, :ns],
                        start=True, stop=False,
                        skip_group_check=True,
                        tile_position=(0, 32 * s),
                    )
                nc.tensor.matmul(zp[:, :ns], wrep_sb[:],
                                 hself_sb[:, :ns],
                                 start=False, stop=True,
                                 skip_group_check=True)
                z_sb = zpool.tile([128, SB], dt.bfloat16, tag="z")
                if len(z_sbs) % 4 == 0:
                    nc.scalar.activation(z_sb[:, :ns], zp[:, :ns], RELU,
                                         bias=bz_sb[:], scale=1.0)
                else:
                    nc.vector.tensor_scalar(
                        z_sb[:, :ns], zp[:, :ns], bz_sb[:], 0.0,
                        mybir.AluOpType.add, mybir.AluOpType.max)
                z_sbs.append(z_sb)

            def emit_wa(w):
                # final contraction: 4 serial accumulating matmuls place
                # neighbor j of the 8-neighbor group t on psum row 8t+j
                z_sbs, ns = w
                wa_ps = ppool.tile([32, SB], dt.float32, tag="wa", bufs=2,
                                   name="wa_ps")
                for t in range(4):
                    nc.tensor.matmul(wa_ps[:, :ns],
                                     wg_sb[:, t, :], z_sbs[t][:, :ns],
                                     start=(t == 0), stop=(t == 3),
                                     skip_group_check=True)
                return wa_ps

            def emit_out(o):
                # psum -> bf16 sbuf -> HBM in [neighbor, node] layout;
                # the host does the final node-major transpose.
                wa_ps, on0, ons = o
                was_sb = wpool.tile([32, SB], dt.bfloat16, tag="was")
                nc.vector.tensor_copy(out=was_sb[:, :ons], in_=wa_ps[:, :ons])
                nc.scalar.dma_start(out=outv[:, on0: on0 + ons],
                                    in_=was_sb[:, :ons])

            for n0, ns in SBS:
                # -- inputs for this superblock (pre-transposed on host),
                #    split into 4 chunks of 8 neighbors alternating between
                #    the sync and scalar DMA queues --
                x2_sb = xpool.tile([128, SB], dt.bfloat16, tag="x2")
                nc.sync.dma_start(
                    out=x2_sb[:, :ns],
                    in_=x2t[128 * n0: 128 * (n0 + ns)].rearrange(
                        "(f n) -> f n", f=128),
                )
                x_sb = xpool.tile([128, N * SB], dt.float8e3, tag="x")
                xt_sb = xt[128 * N * n0: 128 * N * (n0 + ns)].rearrange(
                    "(f j n) -> f j n", f=128, j=N)
                for ci, (j0, j1) in enumerate([(0, 8), (8, 16), (16, 24), (24, 32)]):
                    eng = nc.sync if ci % 2 == 0 else nc.scalar
                    eng.dma_start(
                        out=x_sb[:, j0 * ns: j1 * ns].rearrange(
                            "p (j n) -> p j n", j=j1 - j0),
                        in_=xt_sb[:, j0: j1, :],
                    )

                if first:
                    # HAM warm-up: ~3.5us of dense matmul right after the
                    # first DMA lands, so the PE clock-gate opens to 2.4GHz
                    # before the real stream starts.
                    first = False
                    warm = ppool.tile([128, SB], dt.float32, tag="hp",
                                      bufs=4)
                    for _ in range(8):
                        nc.tensor.matmul(warm[0:H1, :ns], wnb_sb[:],
                                         x2_sb[:, :ns], start=True, stop=True)

                # -- self path: h_self = relu(W_self.T @ x2T + b_self) --
                hs_psum = ppool.tile([128, SB], dt.float32, tag="hp",
                                     bufs=4)
                nc.tensor.matmul(hs_psum[0:H1, :ns], wself_sb[:],
                                 x2_sb[:, :ns], start=True, stop=True)
                hself_sb = hpool.tile([H1, SB], dt.bfloat16, tag="hself")
                nc.scalar.activation(hself_sb[:, :ns], hs_psum[0:H1, :ns],
                                     RELU, bias=bself_sb[:], scale=1.0)

                z_sbs = []
                # mm1: one neighbor pair per psum tile (2-way column-tile
                # concurrency), relu+cast split across the scalar and
                # vector engines.  mm2/wa/store for older blocks
                # interleave at fixed points so every engine stays ~2
                # blocks behind mm1.
                pair_h = []
                for pi in range(16):
                    hp = ppool.tile([128, SB], dt.float32, tag="hp",
                                    bufs=4)
                    for c in range(2):
                        j = 2 * pi + c
                        nc.tensor.matmul(
                            hp[H1 * c: H1 * (c + 1), :ns],
                            wnb_sb[:],
                            x_sb[:, j * ns: (j + 1) * ns],
                            start=True, stop=True,
                            tile_position=(0, H1 * c),
                        )
                    h_sb = hpool.tile([128, SB], dt.bfloat16, tag="h")
                    if pi % 16 in (0, 2, 4, 6, 8, 10, 12, 14, 15):
                        nc.scalar.activation(h_sb[:, :ns], hp[:, :ns],
                                             RELU, bias=bnb_sb[:],
                                             scale=1.0)
                    else:
                        nc.vector.tensor_scalar(
                            h_sb[:, :ns], hp[:, :ns],
                            bnb_sb[:], 0.0,
                            mybir.AluOpType.add, mybir.AluOpType.max)
                    pair_h.append(h_sb)

                    if pi % 4 == 3:
                        pend_blk.append(
                            (pair_h[-4:], ns, z_sbs, hself_sb))
                        if len(pend_blk) > 2:
                            flush_blk(pend_blk.pop(0))

                    if pi == 7 and pend_wa is not None:
                        pend_out = (emit_wa(pend_wa), pend_wa_n0, pend_wa_ns)
                        pend_wa = None
                    if pi == 11 and pend_out is not None:
                        emit_out(pend_out)
                        pend_out = None

                pend_wa = (z_sbs, ns)
                pend_wa_n0, pend_wa_ns = n0, ns

            # drain the pipeline
            while pend_blk:
                flush_blk(pend_blk.pop(0))
            if pend_wa is not None:
                pend_out = (emit_wa(pend_wa), pend_wa_n0, pend_wa_ns)
            emit_out(pend_out)

    nc.compile()
    return nc


def _prep_weights(W_nb, b_nb, W_self, b_self, W_a1, b_a1, W_a2, b_a2):
    """Pack the dense weights into the layouts the kernel expects."""
    W_a1a = W_a1[:H1]          # [64, 16]
    W_a1b = W_a1[H1:]          # [64, 16]

    # mm2 stationary: block-diagonal so one matmul emits z for the two
    # K-stacked neighbors of an h tile.
    w2s = np.zeros((128, 32), np.float32)
    w2s[:H1, :H2] = W_a1a
    w2s[H1:, H2:] = W_a1a

    # self path replicated into all 8 16-column slots
    wrep = np.zeros((H1, 128), np.float32)
    for s in range(4):
        wrep[:, 32 * s: 32 * s + H2] = W_a1b
        wrep[:, 32 * s + 16: 32 * s + 16 + H2] = W_a1b

    # final contraction: z tile t holds neighbors 8t..8t+7; neighbor
    # 8t+2s sits on partitions 32s..32s+15, 8t+2s+1 on 32s+16..32s+31.
    # Place neighbor j on output row 8t+j mod 8 -> global row 8t+...
    wg = np.zeros((4, 128, 32), np.float32)
    for t in range(4):
        for s in range(4):
            wg[t, 32 * s: 32 * s + H2, 8 * t + 2 * s] = W_a2[:, 0]
            wg[t, 32 * s + 16: 32 * s + 16 + H2, 8 * t + 2 * s + 1] = W_a2[:, 0]

    bnb = np.concatenate([b_nb, b_nb]).reshape(128, 1).astype(np.float32)
    bselfv = b_self.reshape(H1, 1).astype(np.float32)
    bzv = np.zeros((128, 1), np.float32)
    for s in range(4):
        bzv[32 * s: 32 * s + H2, 0] = b_a1
        bzv[32 * s + 16: 32 * s + 16 + H2, 0] = b_a1

    return {
        "wnb": np.asarray(W_nb, np.float32).astype(BF16),
        "wself": np.asarray(W_self, np.float32).astype(BF16),
        "w2s": w2s.astype(BF16),
        "wrep": wrep.astype(BF16),
        "wg": wg.astype(BF16),
        "bnb": bnb,
        "bself": bselfv,
        "bz": bzv,
    }


def _prep_core_inputs(x_core, x2_core):
    """Pad to B_PAD nodes, cast (fp8 for x1, bf16 for x2), pre-transpose
    to [F, nbr, node] per superblock, and flatten."""
    xp = np.zeros((B_PAD, N, F), E3M4)
    xp[:B_SH] = x_core.astype(E3M4)
    x2p = np.zeros((B_PAD, F), BF16)
    x2p[:B_SH] = x2_core.astype(BF16)

    xt_parts = []
    x2t_parts = []
    for n0, ns in SBS:
        xt_parts.append(np.ascontiguousarray(
            xp[n0:n0 + ns].transpose(2, 1, 0)).reshape(-1))
        x2t_parts.append(np.ascontiguousarray(
            x2p[n0:n0 + ns].transpose(1, 0)).reshape(-1))
    return np.concatenate(xt_parts), np.concatenate(x2t_parts)


def kernel(input1, input2, W_nb, b_nb, W_self, b_self, W_a1, b_a1, W_a2, b_a2):
    global last_results
    if "nc" not in _cache:
        _cache["nc"] = _build_graph()
    nc = _cache["nc"]

    input1 = np.asarray(input1, np.float32)
    input2 = np.asarray(input2, np.float32)
    b_a2 = np.asarray(b_a2, np.float32)
    wmap = _prep_weights(
        np.asarray(W_nb, np.float32), np.asarray(b_nb, np.float32),
        np.asarray(W_self, np.float32), np.asarray(b_self, np.float32),
        np.asarray(W_a1, np.float32), np.asarray(b_a1, np.float32),
        np.asarray(W_a2, np.float32), b_a2)

    in_maps = []
    for c in range(N_CORES):
        xt_c, x2t_c = _prep_core_inputs(
            input1[c * B_SH: (c + 1) * B_SH],
            input2[c * B_SH: (c + 1) * B_SH])
        m = dict(wmap)
        m["xt"] = xt_c
        m["x2t"] = x2t_c
        in_maps.append(m)

    res = run_bass_kernel_spmd(nc, in_maps, core_ids=list(range(N_CORES)),
                               trace=TRACE)
    last_results = res

    out = np.empty((B * N, 1), np.float32)
    for c in range(N_CORES):
        # device output is [neighbor, node]; transpose to node-major here
        arr = res.results[c]["out"].reshape(32, B_PAD)[:, :B_SH]
        out[c * R_SH: (c + 1) * R_SH, 0] = (
            arr.astype(np.float32).T.reshape(-1) + b_a2[0])
    return out
